# revision 36
# baseline (speedup 1.0000x reference)
"""Bass/Tile kernel for nn_BidirectionalAttention on 8 trn2 NeuronCores.

Sharding: data-parallel over batch (4) x tensor-parallel over head groups
(2). Core c = 2*b + g handles batch b, heads [8g..8g+8) (columns
[512g..512(g+1)) of wq/wk/wv, rows of wo). Each core produces a partial
output projection; the host sums the two head-group partials per batch and
adds bo + bv@wo (the v-bias commutes out of the softmax-weighted average).

Device dataflow (per core, all matmuls in f32r):
  xT [c,s] (host-transposed) --PE--> qT,kT [d,s] (+bias on DVE evac)
  xT, wv                     --PE--> v [s,d] (key-mask applied on evac)
  S^T[k,q] = kT_h^T qT_h  -- head pairs sit in opposite PE row halves so
  consecutive matmuls overlap in the array.
  probs = exp(S^T/8) on ACT (f32r, PSUM->SBUF; no max-subtraction needed
  at this scale, and masked keys are handled multiplicatively below).
  attnT_aug[65,q] = [v_h | m01]^T probs  (rowsum rides as the 65th row;
  masked keys drop out of numerator and denominator together)
  attnT = attnT_aug[:64] * recip(rowsum) --PE--> out partial [q, H].

Schedule: x/wv/wq0/wk0 stream first; pair-0 q/k projections and the first
half of the v-projection run as j-outer accumulation groups across all 8
PSUM banks so the PE tracks DMA arrival; exp starts ~30us in and the
remaining projections + split output projection hide under the exp stream
(ACT ~68us busy is the critical engine). Timeline-sim: ~139us/core.
"""

import sys

sys.path.insert(0, "/opt/trn_rl_repo")
import numpy as np

B, S, H = 4, 1024, 1024
NH, HD = 16, 64
NCORES, GROUPS = 8, 2
DSH = H // GROUPS  # 512 shard width
NHL = NH // GROUPS  # 8 local heads
CT = H // 128  # 8 contraction tiles
ST = S // 128  # 8 s/k tiles
MT = DSH // 128  # 4 d' tiles (= head pairs)
QBS = 512
QB = S // QBS  # 2 q blocks
KTC = ST // 2  # 4 k-chunks of 2 tiles

_cache: dict = {}


def _body(tc, nc, mybir, d, phases=frozenset({"load", "proj", "attn", "out"})):
    import concourse.bass as bass

    F32 = mybir.dt.float32
    F32R = mybir.dt.float32r
    EXP = mybir.ActivationFunctionType.Exp
    from contextlib import ExitStack

    with ExitStack() as ctx:
        persist = ctx.enter_context(tc.tile_pool(name="persist", bufs=1))
        xtp = ctx.enter_context(tc.tile_pool(name="xtp", bufs=1))
        wpool = ctx.enter_context(tc.tile_pool(name="wpool", bufs=1))
        evac = ctx.enter_context(tc.tile_pool(name="evac", bufs=3))
        smalls = ctx.enter_context(tc.tile_pool(name="smalls", bufs=3))
        probs = ctx.enter_context(tc.tile_pool(name="probs", bufs=3))
        # PSUM: "sc" 2x[128,1024] (4 banks) + "ps" 2x[128,512] (2) +
        # "pvA"/"pvB" 1x[128,512]-sized each (2) = 8 banks exactly.
        scp = ctx.enter_context(tc.tile_pool(name="scp", bufs=2, space="PSUM"))
        psp = ctx.enter_context(tc.tile_pool(name="psp", bufs=2, space="PSUM"))
        pvp = ctx.enter_context(tc.tile_pool(name="pvp", bufs=2, space="PSUM"))

        wq_sb = persist.tile([128, CT, DSH], F32R)
        wk_sb = persist.tile([128, CT, DSH], F32R)
        qt_sb = persist.tile([128, MT, S], F32R)
        kt_sb = persist.tile([128, MT, S], F32R)
        at_sb = persist.tile([128, MT, S], F32R)
        v_sb = persist.tile([128, ST, NHL, HD + 1], F32R)
        bq_sb = persist.tile([128, MT], F32)
        bk_sb = persist.tile([128, MT], F32)
        m01_sb = persist.tile([128, ST], F32)
        xt_sb = xtp.tile([128, CT, S], F32R)
        # wv and wo share one slot: wo streams into wv's space after the
        # v-projection consumed wv (both 16 KB/partition).
        wv_sb = wpool.tile([128, CT, DSH], F32R, tag="w")

        def qk_proj(m):
            """q/k projection for head-pair m from resident xt/wq/wk."""
            for w_sb, b_sb, o_sb in (
                (wq_sb, bq_sb, qt_sb),
                (wk_sb, bk_sb, kt_sb),
            ):
                for n in range(QB):
                    ps = psp.tile([128, QBS], F32, tag="ps")
                    for j in range(CT):
                        nc.tensor.matmul(
                            ps[:],
                            w_sb[:, j, m * 128 : (m + 1) * 128],
                            xt_sb[:, j, n * QBS : (n + 1) * QBS],
                            start=(j == 0),
                            stop=(j == CT - 1),
                        )
                    nc.vector.tensor_scalar_add(
                        o_sb[:, m, n * QBS : (n + 1) * QBS],
                        ps[:],
                        b_sb[:, m : m + 1],
                    )

        def granule_scores_chunk(m, qb, c):
            scA = scp.tile([128, 2 * QBS], F32, tag="sc")
            scB = scp.tile([128, 2 * QBS], F32, tag="sc")
            for half in range(2):
                kt_i = 2 * c + half
                for sc, hb in ((scA, 0), (scB, 64)):
                    nc.tensor.matmul(
                        sc[:, half * QBS : (half + 1) * QBS],
                        kt_sb[hb : hb + 64, m, kt_i * 128 : (kt_i + 1) * 128],
                        qt_sb[hb : hb + 64, m, qb * QBS : (qb + 1) * QBS],
                        start=True,
                        stop=True,
                    )
            prA = probs.tile([128, 2, QBS], F32R, tag="prA")
            prB = probs.tile([128, 2, QBS], F32R, tag="prB")
            nc.scalar.activation(
                prA[:].rearrange("p a b -> p (a b)"), scA[:], EXP, scale=0.125
            )
            nc.scalar.activation(
                prB[:].rearrange("p a b -> p (a b)"), scB[:], EXP, scale=0.125
            )
            return (prA, prB)

        def granule_scores(m, qb):
            """scores + exp for one head pair; returns probs chunk tiles."""
            return [granule_scores_chunk(m, qb, c) for c in range(KTC)]

        def granule_pv_chunk(pvA, pvB, prs, m, c):
            prA, prB = prs[c]
            for half in range(2):
                kt_i = 2 * c + half
                for pv, pr, h in ((pvA, prA, 2 * m), (pvB, prB, 2 * m + 1)):
                    nc.tensor.matmul(
                        pv[:],
                        v_sb[:, kt_i, h, :],
                        pr[:, half, :],
                        start=(kt_i == 0),
                        stop=(kt_i == ST - 1),
                    )

        def granule_norm(pvA, pvB, m, qb):
            for pv, hb in ((pvA, 0), (pvB, 64)):
                # fast evac to SBUF frees the PSUM slot for the next granule
                pvs = smalls.tile([HD + 1, QBS], F32, tag="pvs")
                nc.vector.tensor_copy(pvs[:], pv[:])
                r = smalls.tile([1, QBS], F32, tag="r")
                nc.vector.reciprocal(r[:], pvs[HD : HD + 1, :])
                rb = smalls.tile([64, QBS], F32, tag="rb")
                nc.gpsimd.partition_broadcast(rb[:], r[:])
                nc.vector.tensor_mul(
                    at_sb[hb : hb + 64, m, qb * QBS : (qb + 1) * QBS],
                    pvs[0:HD, :],
                    rb[:],
                )

        def granule_pv(m, qb, prs):
            """PV + normalize. MUST be emitted after the v-projection."""
            pvA = pvp.tile([HD + 1, QBS], F32, tag="pv")
            pvB = pvp.tile([HD + 1, QBS], F32, tag="pv")
            for c in range(KTC):
                granule_pv_chunk(pvA, pvB, prs, m, c)
            granule_norm(pvA, pvB, m, qb)

        def qk_group(m, proj, n):
            w_sb, b_sb, o_sb = (
                (wq_sb, bq_sb, qt_sb) if proj == 0 else (wk_sb, bk_sb, kt_sb)
            )
            ps = psp.tile([128, QBS], F32, tag="ps")
            for j in range(CT):
                nc.tensor.matmul(
                    ps[:],
                    w_sb[:, j, m * 128 : (m + 1) * 128],
                    xt_sb[:, j, n * QBS : (n + 1) * QBS],
                    start=(j == 0),
                    stop=(j == CT - 1),
                )
            nc.vector.tensor_scalar_add(
                o_sb[:, m, n * QBS : (n + 1) * QBS], ps[:], b_sb[:, m : m + 1]
            )

        def attn_granule(m, qb):
            granule_pv(m, qb, granule_scores(m, qb))

        def out_proj(wo_sb, mq, pool=None, tag="ps"):
            for n in range(QB):
                ps = (pool or psp).tile([128, QBS], F32, tag=tag)
                for j in range(MT):
                    nc.tensor.matmul(
                        ps[:],
                        at_sb[:, j, mq * 128 : (mq + 1) * 128],
                        wo_sb[:, j, n * QBS : (n + 1) * QBS],
                        start=(j == 0),
                        stop=(j == MT - 1),
                    )
                o = evac.tile([128, QBS], F32, tag="o")
                nc.vector.tensor_copy(o[:], ps[:])
                nc.sync.dma_start(
                    d["outp"][mq * 128 : (mq + 1) * 128, n * QBS : (n + 1) * QBS],
                    o[:],
                )

        # -------- loads, in DMA priority order: x + wv + pair-0 columns
        # of wq/wk stream first (they gate both the exp start and the
        # v-readiness); remaining wq/wk columns follow.
        for j in range(CT):
            nc.sync.dma_start(xt_sb[:, j, :], d["xt"][j * 128 : (j + 1) * 128, :])
            nc.sync.dma_start(wv_sb[:, j, :], d["wv"][j * 128 : (j + 1) * 128, :])
            nc.sync.dma_start(
                wq_sb[:, j, 0:128], d["wq"][j * 128 : (j + 1) * 128, 0:128]
            )
            nc.sync.dma_start(
                wk_sb[:, j, 0:128], d["wk"][j * 128 : (j + 1) * 128, 0:128]
            )
        nc.sync.dma_start(bq_sb[:], d["bq"])
        nc.sync.dma_start(bk_sb[:], d["bk"])
        nc.sync.dma_start(m01_sb[:], d["m01"])
        nc.sync.dma_start(v_sb[:, :, :, HD : HD + 1], d["m01c"])
        for j in range(CT):
            nc.sync.dma_start(
                wq_sb[:, j, 128:256], d["wq"][j * 128 : (j + 1) * 128, 128:256]
            )
            nc.sync.dma_start(
                wk_sb[:, j, 128:256], d["wk"][j * 128 : (j + 1) * 128, 128:256]
            )
        for j in range(CT):
            nc.sync.dma_start(
                wq_sb[:, j, 256:DSH], d["wq"][j * 128 : (j + 1) * 128, 256:DSH]
            )
            nc.sync.dma_start(
                wk_sb[:, j, 256:DSH], d["wk"][j * 128 : (j + 1) * 128, 256:DSH]
            )
        if "proj" not in phases:
            return

        def v_group(m, tag):
            ps = (pvp if tag == "pv" else psp).tile([128, QBS], F32, tag=tag)
            for j in range(CT):
                nc.tensor.matmul(
                    ps[:],
                    xt_sb[:, j, m * 128 : (m + 1) * 128],
                    wv_sb[:, j, :],
                    start=(j == 0),
                    stop=(j == CT - 1),
                )
            return ps

        def v_evac(m, ps):
            nc.vector.tensor_scalar_mul(
                v_sb[:, m, :, 0:HD],
                ps[:].rearrange("p (h e) -> p h e", h=NHL),
                m01_sb[:, m : m + 1],
            )

        # -------- phase L: pair-0 q/k projections (4 groups on the score
        # banks) and the first 4 v groups (ps/pv banks), all j-outer so the
        # PE tracks DMA arrival across all 8 PSUM banks.
        g_sc1 = scp.tile([128, 2, QBS], F32, tag="sc")
        g_sc2 = scp.tile([128, 2, QBS], F32, tag="sc")
        qk0_groups = [
            (g_sc1[:, 0, :], wq_sb, 0, 0),
            (g_sc2[:, 0, :], wk_sb, 0, 0),
            (g_sc1[:, 1, :], wq_sb, 0, 1),
            (g_sc2[:, 1, :], wk_sb, 0, 1),
        ]
        gv0 = psp.tile([128, QBS], F32, tag="ps")
        gv1 = psp.tile([128, QBS], F32, tag="ps")
        gv2 = pvp.tile([128, QBS], F32, tag="pv")
        gv3 = pvp.tile([128, QBS], F32, tag="pv")
        gv = [gv0, gv1, gv2, gv3]
        for j in range(CT):
            for mv in range(4):
                nc.tensor.matmul(
                    gv[mv][:],
                    xt_sb[:, j, mv * 128 : (mv + 1) * 128],
                    wv_sb[:, j, :],
                    start=(j == 0),
                    stop=(j == CT - 1),
                )
            for g_ps, w_sb, m, n in qk0_groups:
                nc.tensor.matmul(
                    g_ps,
                    w_sb[:, j, m * 128 : (m + 1) * 128],
                    xt_sb[:, j, n * QBS : (n + 1) * QBS],
                    start=(j == 0),
                    stop=(j == CT - 1),
                )
        for mv in range(4):
            v_evac(mv, gv[mv])
        for g_ps, w_sb, m, n in qk0_groups:
            o_sb, b_sb = (qt_sb, bq_sb) if w_sb is wq_sb else (kt_sb, bk_sb)
            nc.vector.tensor_scalar_add(
                o_sb[:, m, n * QBS : (n + 1) * QBS], g_ps, b_sb[:, m : m + 1]
            )

        if "attn" not in phases:
            # still finish the v projection for ablation builds
            for m in range(4, ST):
                v_evac(m, v_group(m, "ps"))
            wo_sb = wpool.tile([128, MT, H], F32R, tag="w")
            nc.sync.dma_start(wo_sb[:], d["wo"].rearrange("(m p) h -> p m h", p=128))
            return

        # pair-0 scores/exp stream immediately (top ACT priority)
        pr00 = granule_scores(0, 0)
        pr01 = granule_scores(0, 1)

        # PV(0,0) starts on the wave-A v groups while wave B (m4-7, from
        # resident wv) finishes on the ps banks.
        pvA00 = pvp.tile([HD + 1, QBS], F32, tag="pv")
        pvB00 = pvp.tile([HD + 1, QBS], F32, tag="pv")
        granule_pv_chunk(pvA00, pvB00, pr00, 0, 0)
        granule_pv_chunk(pvA00, pvB00, pr00, 0, 1)
        for m in (4, 5):
            v_evac(m, v_group(m, "ps"))
        granule_pv_chunk(pvA00, pvB00, pr00, 0, 2)
        for m in (6, 7):
            v_evac(m, v_group(m, "ps"))
        granule_pv_chunk(pvA00, pvB00, pr00, 0, 3)

        # wo reuses wv's SBUF slot once the v projection consumed wv
        wo_sb = wpool.tile([128, MT, H], F32R, tag="w")
        nc.sync.dma_start(wo_sb[:], d["wo"].rearrange("(m p) h -> p m h", p=128))

        granule_norm(pvA00, pvB00, 0, 0)
        for n in range(QB):
            g1 = psp.tile([128, QBS], F32, tag="ps")
            g2 = psp.tile([128, QBS], F32, tag="ps")
            for j in range(CT):
                nc.tensor.matmul(
                    g1[:], wq_sb[:, j, 128:256],
                    xt_sb[:, j, n * QBS : (n + 1) * QBS],
                    start=(j == 0), stop=(j == CT - 1),
                )
                nc.tensor.matmul(
                    g2[:], wk_sb[:, j, 128:256],
                    xt_sb[:, j, n * QBS : (n + 1) * QBS],
                    start=(j == 0), stop=(j == CT - 1),
                )
            nc.vector.tensor_scalar_add(
                qt_sb[:, 1, n * QBS : (n + 1) * QBS], g1[:], bq_sb[:, 1:2]
            )
            nc.vector.tensor_scalar_add(
                kt_sb[:, 1, n * QBS : (n + 1) * QBS], g2[:], bk_sb[:, 1:2]
            )
        granule_pv(0, 1, pr01)
        attn_granule(1, 0)
        qk_proj(2)
        attn_granule(1, 1)
        qk_proj(3)
        attn_granule(2, 0)
        attn_granule(3, 0)
        if "out" in phases:
            out_proj(wo_sb, 0)
            out_proj(wo_sb, 1)
        attn_granule(2, 1)
        if "out" in phases:
            out_proj(wo_sb, 2)
            out_proj(wo_sb, 3)
        attn_granule(3, 1)
        if "out" in phases:
            for mq in range(ST // 2, ST):
                ps = scp.tile([128, QB, QBS], F32, tag="sc")
                for j in range(MT):
                    for n in range(QB):
                        nc.tensor.matmul(
                            ps[:, n, :],
                            at_sb[:, j, mq * 128 : (mq + 1) * 128],
                            wo_sb[:, j, n * QBS : (n + 1) * QBS],
                            start=(j == 0),
                            stop=(j == MT - 1),
                        )
                o = evac.tile([128, H], F32, tag="ow")
                nc.vector.tensor_copy(o[:], ps[:].rearrange("p a b -> p (a b)"))
                nc.sync.dma_start(
                    d["outp"][mq * 128 : (mq + 1) * 128, :], o[:]
                )


def _build(phases=frozenset({'load','proj','attn','out'})):
    key = ('nc', tuple(sorted(phases)))
    if key in _cache:
        return _cache[key]
    import concourse.tile as tile
    from concourse import bacc, mybir

    F32 = mybir.dt.float32
    F32R = mybir.dt.float32r
    nc = bacc.Bacc(
        "TRN2", target_bir_lowering=False, debug=False, num_devices=NCORES
    )
    d = {
        "xt": nc.dram_tensor("xt", [H, S], F32R, kind="ExternalInput").ap(),
        "wq": nc.dram_tensor("wq", [H, DSH], F32R, kind="ExternalInput").ap(),
        "wk": nc.dram_tensor("wk", [H, DSH], F32R, kind="ExternalInput").ap(),
        "wv": nc.dram_tensor("wv", [H, DSH], F32R, kind="ExternalInput").ap(),
        "wo": nc.dram_tensor("wo", [DSH, H], F32R, kind="ExternalInput").ap(),
        "bq": nc.dram_tensor("bq", [128, MT], F32, kind="ExternalInput").ap(),
        "bk": nc.dram_tensor("bk", [128, MT], F32, kind="ExternalInput").ap(),
        "m01": nc.dram_tensor("m01", [128, ST], F32, kind="ExternalInput").ap(),
        "m01c": nc.dram_tensor(
            "m01c", [128, ST, NHL, 1], F32R, kind="ExternalInput"
        ).ap(),
        "outp": nc.dram_tensor("outp", [S, H], F32, kind="ExternalOutput").ap(),
    }
    with tile.TileContext(nc) as tc:
        _body(tc, nc, mybir, d, phases)
    nc.compile()
    _cache[key] = nc
    return nc


def _in_maps(x, mask, wq, bq, wk, bk, wv, bv, wo, bo):
    maps = []
    for c in range(NCORES):
        b, g = divmod(c, 2)
        sl = slice(g * DSH, (g + 1) * DSH)
        m01 = (~mask[b]).astype(np.float32)  # 1.0 = keep, 0.0 = masked key
        m01_pm = np.ascontiguousarray(m01.reshape(ST, 128).T)
        maps.append(
            {
                "xt": np.ascontiguousarray(x[b].T),
                "wq": np.ascontiguousarray(wq[:, sl]),
                "wk": np.ascontiguousarray(wk[:, sl]),
                "wv": np.ascontiguousarray(wv[:, sl]),
                "wo": np.ascontiguousarray(wo[sl, :]),
                "bq": np.ascontiguousarray(bq[sl].reshape(MT, 128).T),
                "bk": np.ascontiguousarray(bk[sl].reshape(MT, 128).T),
                "m01": m01_pm,
                "m01c": np.ascontiguousarray(
                    np.broadcast_to(m01_pm[:, :, None, None], (128, ST, NHL, 1))
                ),
            }
        )
    return maps


def _get_runner():
    """Build (once) a cached jitted SPMD executable over the 8 cores.

    Replicates bass2jax.run_bass_via_pjrt's multi-core path, but holds on
    to the jitted function so repeat kernel() calls don't re-lower or
    re-run the NEFF compile.
    """
    if "runner" in _cache:
        return _cache["runner"]
    import jax
    from jax.experimental.shard_map import shard_map
    from jax.sharding import Mesh, PartitionSpec
    from concourse import bass2jax, mybir

    nc = _build()
    bass2jax.install_neuronx_cc_hook()
    partition_name = (
        nc.partition_id_tensor.name if nc.partition_id_tensor else None
    )

    in_names, out_names, out_avals, zero_outs = [], [], [], []
    for alloc in nc.m.functions[0].allocations:
        if not isinstance(alloc, mybir.MemoryLocationSet):
            continue
        name = alloc.memorylocations[0].name
        if alloc.kind == "ExternalInput":
            if name != partition_name:
                in_names.append(name)
        elif alloc.kind == "ExternalOutput":
            shape = tuple(alloc.tensor_shape)
            dtype = mybir.dt.np(alloc.dtype)
            out_avals.append(jax.core.ShapedArray(shape, dtype))
            out_names.append(name)
            zero_outs.append(np.zeros(shape, dtype))
    n_params = len(in_names)
    n_outs = len(out_avals)
    all_names = in_names + out_names
    if partition_name is not None:
        all_names = all_names + [partition_name]
    donate = tuple(range(n_params, n_params + n_outs))

    def _body(*args):
        operands = list(args)
        if partition_name is not None:
            operands.append(bass2jax.partition_id_tensor())
        outs = bass2jax._bass_exec_p.bind(
            *operands,
            out_avals=tuple(out_avals),
            in_names=tuple(all_names),
            out_names=tuple(out_names),
            lowering_input_output_aliases=(),
            sim_require_finite=True,
            sim_require_nnan=True,
            nc=nc,
        )
        return tuple(outs)

    devices = jax.devices()[:NCORES]
    mesh = Mesh(np.asarray(devices), ("core",))
    sharded = jax.jit(
        shard_map(
            _body,
            mesh=mesh,
            in_specs=(PartitionSpec("core"),) * (n_params + n_outs),
            out_specs=(PartitionSpec("core"),) * n_outs,
            check_rep=False,
        ),
        donate_argnums=donate,
        keep_unused=True,
    )

    def run(maps):
        concat_in = [
            np.concatenate([np.asarray(m[name]) for m in maps], axis=0)
            for name in in_names
        ]
        concat_zeros = [
            np.zeros((NCORES * z.shape[0], *z.shape[1:]), z.dtype) for z in zero_outs
        ]
        out_arrs = sharded(*concat_in, *concat_zeros)
        return [
            {
                name: np.asarray(out_arrs[i]).reshape(NCORES, *out_avals[i].shape)[c]
                for i, name in enumerate(out_names)
            }
            for c in range(NCORES)
        ]

    _cache["runner"] = run
    return run


def kernel(**inputs):
    np_in = {k: np.asarray(v) for k, v in inputs.items()}
    x = np_in["x"].astype(np.float32, copy=False)
    mask = np_in["mask"].astype(bool, copy=False)
    args = [
        np_in[k].astype(np.float32, copy=False)
        for k in ("wq", "bq", "wk", "bk", "wv", "bv", "wo", "bo")
    ]
    wq, bq, wk, bk, wv, bv, wo, bo = args

    run = _get_runner()
    maps = _in_maps(x, mask, wq, bq, wk, bk, wv, bv, wo, bo)
    results = run(maps)
    outs = [r["outp"] for r in results]
    const_row = bv @ wo + bo  # v-bias folded out of the device kernel
    out = np.stack(
        [outs[2 * b] + outs[2 * b + 1] + const_row[None, :] for b in range(B)]
    )
    return out.astype(np.float32)
